# revision 1
# baseline (speedup 1.0000x reference)
"""Trainium2 Bass kernel for a SegFormer-style transformer block.

Reference computation (per batch element b):
    x  = x + attention(LN1(x))          # 8 heads, d=64, no qkv bias
    x  = x + mixffn(LN2(x))             # fc1 -> dwconv3x3 -> gelu -> fc2

Sharding: pure data-parallel over batch B=8 across the 8 NeuronCores
(one batch element per core, weights replicated, no collectives).

Per-core layout strategy:
  - LayerNorms run token-major ([128 tok, 512 C] tiles, bn_stats over C).
  - Everything else runs feature-major ("transposed": features on
    partitions, tokens on the free axis) so that matmul contraction dims
    land on partitions naturally; PE-transposes (via identity matmul)
    convert between the two layouts.
  - Attention computes S^T = K^T.T @ Q^T per head, exp on ACT (scores are
    small: no max-subtraction needed), and folds the softmax denominator
    into the A@V matmul via a ones-column appended to V.  Normalization
    by 1/denom happens after A@V, before proj.
  - The depthwise 3x3 conv runs on the PE as 9 diagonal-matrix matmuls
    accumulating in PSUM (diag(w_tap) @ shifted_view), one [128ch, 1024sp]
    tile at a time.
  - Matmul dtypes: float32r (full-rate fp32) for qkv/scores/proj/fc1/
    dwconv; bf16 for A@V and fc2 (inputs produced in bf16 to save SBUF).

Self-contained: hardcodes all shapes; takes full inputs, returns full
output.
"""

import numpy as np
import ml_dtypes

import concourse.bass as bass
import concourse.tile as tile
from concourse import bacc, mybir
from concourse import bass_utils
from concourse.bass import ts, ds
from concourse.masks import make_identity

P = 128
NTOK = 1024
C = 512
HID = 2048
NH = 8
D = 64
HH = 32
WW = 32
NT = NTOK // P      # 8 token tiles
CT = C // P         # 4 feature tiles
HCT = HID // P      # 16 hidden tiles
EPS = 1e-5
N_CORES = 8

f32 = mybir.dt.float32
f32r = mybir.dt.float32r
bf16 = mybir.dt.bfloat16
AF = mybir.ActivationFunctionType
OP = mybir.AluOpType

# tap order: center first so the first matmul in each PSUM accumulation
# group covers the full bank region (sets has_written everywhere).
TAP_ORDER = [(1, 1), (0, 0), (0, 1), (0, 2), (1, 0), (1, 2), (2, 0), (2, 1), (2, 2)]


def _emit(tc, d, out_ap, gelu_mode="hw", dbg=None):
    def dump(key, ap):
        if dbg is not None and key in dbg:
            tc.nc.sync.dma_start(dbg[key], ap)

    nc = tc.nc

    # ---- whole-kernel pools ----
    pool_const = tc.alloc_tile_pool(name="const", bufs=1)
    pool_x = tc.alloc_tile_pool(name="x", bufs=1)
    pool_x1 = tc.alloc_tile_pool(name="x1", bufs=1)
    pool_stats = tc.alloc_tile_pool(name="stats", bufs=4)
    pool_out = tc.alloc_tile_pool(name="outp", bufs=3)
    pool_pmm = tc.alloc_tile_pool(name="pmm", bufs=4, space="PSUM")
    pool_pbig = tc.alloc_tile_pool(name="pbig", bufs=2, space="PSUM")

    ident = pool_const.tile([P, P], f32, tag="ident", name="ident")
    make_identity(nc, ident[:])
    zconst = pool_const.tile([P, 1], f32, tag="zconst", name="zconst")
    nc.vector.memset(zconst[:], 0.0)
    nc.const_aps.aps[(f32, 0.0)] = zconst[:]
    epsap = pool_const.tile([P, 1], f32, tag="epsap", name="epsap")
    nc.vector.memset(epsap[:], EPS)

    def pp_load(name, cols, tag):
        t = pool_const.tile([P, cols], f32, tag=tag)
        nc.sync.dma_start(t[:], d[name].rearrange("(o p) -> p o", p=P))
        return t

    g1pp = pp_load("ln1_g", CT, "g1")
    b1pp = pp_load("ln1_b", CT, "b1")
    g2pp = pp_load("ln2_g", CT, "g2")
    b2pp = pp_load("ln2_b", CT, "b2")
    pbpp = pp_load("proj_b", CT, "pb")
    f1bpp = pp_load("fc1_b", HCT, "f1b")
    dwbpp = pp_load("dw_b", HCT, "dwb")
    f2bpp = pp_load("fc2_b", CT, "f2b")
    dwpp = pool_const.tile([P, HCT, 9], f32, tag="dww", name="dww")
    nc.sync.dma_start(dwpp[:], d["dw_w9"].rearrange("(t p) k -> p t k", p=P))

    x_sb = []
    for i in range(NT):
        t = pool_x.tile([P, C], f32, tag=f"x{i}", name=f"x{i}")
        nc.sync.dma_start(t[:], d["x"][ts(i, P), :])
        x_sb.append(t)
    x1_sb = [pool_x1.tile([P, C], f32, tag=f"x1_{i}", name=f"x1_{i}") for i in range(NT)]

    def emit_ln(src_tiles, gpp, bpp, dstT_tiles, pool_xn):
        """Token-major LN over C, PE-transpose into feature-major dstT."""
        for i in range(NT):
            st6 = pool_stats.tile([P, 6], f32, tag="st6", name="st6")
            nc.vector.bn_stats(st6[:], src_tiles[i][:])
            mv = pool_stats.tile([P, 2], f32, tag="mv", name="mv")
            nc.vector.bn_aggr(mv[:], st6[:])
            sd = pool_stats.tile([P, 1], f32, tag="sd", name="sd")
            nc.scalar.activation(sd[:], mv[:, 1:2], AF.Sqrt, bias=epsap[:, 0:1])
            rstd = pool_stats.tile([P, 1], f32, tag="rstd", name="rstd")
            nc.vector.reciprocal(rstd[:], sd[:])
            nb = pool_stats.tile([P, 1], f32, tag="nb", name="nb")
            nc.vector.scalar_tensor_tensor(
                nb[:], mv[:, 0:1], -1.0, rstd[:], OP.mult, OP.mult
            )
            xn = pool_xn.tile([P, C], f32, tag="xn", name="xn")
            nc.scalar.activation(
                xn[:], src_tiles[i][:], AF.Identity, bias=nb[:], scale=rstd[:]
            )
            for c in range(CT):
                pt = pool_pmm.tile([P, P], f32, tag="mm", name="mm")
                nc.tensor.transpose(pt[:], xn[:, ts(c, P)], ident[:])
                nc.vector.tensor_scalar(
                    dstT_tiles[c][:, ts(i, P)],
                    pt[:],
                    gpp[:, c : c + 1],
                    bpp[:, c : c + 1],
                    OP.mult,
                    OP.add,
                )

    # ================= S0-S3: LN1, QKV =================
    pool_a = tc.alloc_tile_pool(name="poolA", bufs=2)
    wq_sb = []
    for ci in range(CT):
        t = pool_a.tile([P, 3 * C], bf16, tag=f"wq{ci}", name=f"wq{ci}")
        nc.sync.dma_start(t[:], d["qkv_wTb"][ts(ci, P), :])
        wq_sb.append(t)
    xlnT = [pool_a.tile([P, NTOK], bf16, tag=f"xlnT{c}", name=f"xlnT{c}") for c in range(CT)]

    emit_ln(x_sb, g1pp, b1pp, xlnT, pool_a)

    dump("xlnT0", xlnT[0][:])
    pool_b = tc.alloc_tile_pool(name="poolB", bufs=1, side="right")
    Q_sb = [pool_b.tile([P, NTOK], bf16, tag=f"Q{t}", name=f"Q{t}") for t in range(NH // 2)]
    K_sb = [pool_b.tile([P, NTOK], bf16, tag=f"K{t}", name=f"K{t}") for t in range(NH // 2)]
    V_sb = [pool_b.tile([P, NH, D + 1], bf16, tag=f"V{j}", name=f"V{j}") for j in range(NT)]
    E_sb = [pool_b.tile([P, NTOK], bf16, tag=f"E{j}", name=f"E{j}") for j in range(NT)]
    for j in range(NT):
        nc.vector.memset(V_sb[j][:], 1.0)

    # Q/K head-pair chunks (qkv_wT is host-reordered: [Qpairs | Kpairs | V]):
    # psum rows 0-63 = head 2t, 64-127 = head 2t+1.
    for t in range(NH // 2):
        for dst, base in ((Q_sb, 0), (K_sb, C)):
            for nqc in range(2):
                ps = pool_pmm.tile([P, 512], f32, tag="mm", name="mm")
                for ci in range(CT):
                    nc.tensor.matmul(
                        ps[:],
                        wq_sb[ci][:, base + t * P : base + (t + 1) * P],
                        xlnT[ci][:, ts(nqc, 512)],
                        start=(ci == 0),
                        stop=(ci == CT - 1),
                    )
                nc.scalar.copy(dst[t][:, ts(nqc, 512)], ps[:])

    # V in token-major: [tok, (h, dv)]
    for j in range(NT):
        ps = pool_pmm.tile([P, 512], f32, tag="mm", name="mm")
        for ci in range(CT):
            nc.tensor.matmul(
                ps[:],
                xlnT[ci][:, ts(j, P)],
                wq_sb[ci][:, 2 * C : 3 * C],
                start=(ci == 0),
                stop=(ci == CT - 1),
            )
        nc.scalar.copy(
            V_sb[j][:, :, 0:D], ps[:].rearrange("p (h r) -> p h r", h=NH)
        )

    dump("Q0", Q_sb[0][:])
    dump("K0", K_sb[0][:])
    dump("V0", V_sb[0][:])
    pool_a.release()

    # ================= S4: attention per head =================
    pool_c = tc.alloc_tile_pool(name="poolC", bufs=1)
    OT_sb = [pool_c.tile([D, NTOK], bf16, tag=f"OT{h}", name=f"OT{h}") for h in range(NH)]
    pw_sb = []
    for h in range(NH):
        t = pool_c.tile([D, C], bf16, tag=f"pw{h}", name=f"pw{h}")
        nc.sync.dma_start(t[:], d["proj_wTb"][ts(h, D), :])
        pw_sb.append(t)
    PT_sb = [pool_c.tile([P, NTOK], f32, tag=f"PT{c}", name=f"PT{c}") for c in range(CT)]
    pool_r = tc.alloc_tile_pool(name="poolR", bufs=2)

    for h in range(NH):
        t, o = h // 2, (h % 2) * D
        q_ap = Q_sb[t][o : o + D, :]
        k_ap = K_sb[t][o : o + D, :]
        for j in range(NT):
            for nqc in range(2):
                ps = pool_pmm.tile([P, 512], f32, tag="mm", name="mm")
                nc.tensor.matmul(ps[:], k_ap[:, ts(j, P)], q_ap[:, ts(nqc, 512)])
                nc.scalar.activation(E_sb[j][:, ts(nqc, 512)], ps[:], AF.Exp)
        ps_o = pool_pbig.tile([P, NTOK], f32, tag="big", name="big")
        for nqc in range(2):
            for j in range(NT):
                nc.tensor.matmul(
                    ps_o[0 : D + 1, ts(nqc, 512)],
                    V_sb[j][:, h, :],
                    E_sb[j][:, ts(nqc, 512)],
                    start=(j == 0),
                    stop=(j == NT - 1),
                )
        if h == 0:
            dump("E0", E_sb[0][:])
        # partition_broadcast only reads physical partition 0, so move the
        # denominator row there first (ACT evacuates the PSUM row in place,
        # then a SBUF->SBUF DMA shifts it to partition 0; DVE cannot shift).
        rt = pool_r.tile([D + 1, NTOK], f32, tag="rt", name="rt")
        nc.vector.reciprocal(rt[D : D + 1, :], ps_o[D : D + 1, :])
        rt0 = pool_r.tile([1, NTOK], f32, tag="rt0", name="rt0")
        nc.sync.dma_start(rt0[:], rt[D : D + 1, :])
        Rh = pool_r.tile([D, NTOK], f32, tag="Rh", name="Rh")
        nc.gpsimd.partition_broadcast(Rh[:], rt0[:])
        nc.vector.tensor_mul(OT_sb[h][:], ps_o[0:D, :], Rh[:])
        if h == 0:
            dump("Rh0", Rh[:])
            dump("OT0", OT_sb[0][:])

    pool_r.release()
    pool_b.release()

    # ================= S5: proj =================
    for oc in range(CT):
        for nqc in range(2):
            ps = pool_pmm.tile([P, 512], f32, tag="mm", name="mm")
            for h in range(NH):
                nc.tensor.matmul(
                    ps[:],
                    pw_sb[h][:, ts(oc, P)],
                    OT_sb[h][:, ts(nqc, 512)],
                    start=(h == 0),
                    stop=(h == NH - 1),
                )
            nc.scalar.activation(
                PT_sb[oc][:, ts(nqc, 512)], ps[:], AF.Identity,
                bias=pbpp[:, oc : oc + 1],
            )

    dump("PT0", PT_sb[0][:])
    # ================= S6: residual 1 (transpose back) =================
    for i in range(NT):
        for c in range(CT):
            pt = pool_pmm.tile([P, P], f32, tag="mm", name="mm")
            nc.tensor.transpose(pt[:], PT_sb[c][:, ts(i, P)], ident[:])
            nc.vector.tensor_add(x1_sb[i][:, ts(c, P)], pt[:], x_sb[i][:, ts(c, P)])

    pool_c.release()

    dump("x1_0", x1_sb[0][:])
    # ================= S7: LN2 =================
    pool_d = tc.alloc_tile_pool(name="poolD", bufs=2)
    x2T = [pool_d.tile([P, NTOK], bf16, tag=f"x2T{c}", name=f"x2T{c}") for c in range(CT)]
    w1_sb = []
    for ci in range(CT):
        t = pool_d.tile([P, HID], bf16, tag=f"w1{ci}", name=f"w1{ci}")
        nc.sync.dma_start(t[:], d["fc1_wTb"][ts(ci, P), :])
        w1_sb.append(t)
    emit_ln(x1_sb, g2pp, b2pp, x2T, pool_d)

    dump("x2T0", x2T[0][:])
    # ================= S8+S9: fc1 + dwconv + gelu (fused per tile) ====
    pool_e = tc.alloc_tile_pool(name="poolE", bufs=1, side="right")
    G_sb = [pool_e.tile([P, NTOK], bf16, tag=f"G{hc}", name=f"G{hc}") for hc in range(HCT)]
    w2_sb = []
    for hc in range(HCT):
        t = pool_e.tile([P, C], bf16, tag=f"w2{hc}", name=f"w2{hc}")
        nc.sync.dma_start(t[:], d["fc2_wTb"][ts(hc, P), :])
        w2_sb.append(t)
    pool_ht = tc.alloc_tile_pool(name="poolHT", bufs=3, side="right")
    pool_dg = tc.alloc_tile_pool(name="poolDG", bufs=4, side="right")

    for hc in range(HCT):
        # HT is x-zero-padded: [P, 32, 34] with zero columns at x=0 and 33,
        # so every conv tap writes a flat contiguous PSUM row range.
        ht = pool_ht.tile([P, HH, WW + 2], bf16, tag="HT", name="HT")
        nc.vector.memset(ht[:, :, 0:1], 0.0)
        nc.vector.memset(ht[:, :, WW + 1 : WW + 2], 0.0)
        for nqc in range(2):
            ps = pool_pmm.tile([P, 512], f32, tag="mm", name="mm")
            for ci in range(CT):
                nc.tensor.matmul(
                    ps[:],
                    w1_sb[ci][:, ts(hc, P)],
                    x2T[ci][:, ts(nqc, 512)],
                    start=(ci == 0),
                    stop=(ci == CT - 1),
                )
            nc.scalar.activation(
                ht[:, 16 * nqc : 16 * nqc + 16, 1 : WW + 1],
                ps[:].rearrange("p (y x) -> p y x", x=WW),
                AF.Identity,
                bias=f1bpp[:, hc : hc + 1],
            )
        # depthwise 3x3 conv: 9 diagonal matmuls per PSUM bank, taps
        # emitted bank-interleaved so each diag tile has a short lifetime.
        pd = pool_pbig.tile([P, NTOK], f32, tag="big", name="big")
        ntaps = len(TAP_ORDER)
        for n, (ky, kx) in enumerate(TAP_ORDER):
            dy, dx = ky - 1, kx - 1
            dg = pool_dg.tile([P, P], bf16, tag="dg", name="dg")
            nc.vector.tensor_scalar_mul(
                dg[:], ident[:], dwpp[:, hc, 3 * ky + kx : 3 * ky + kx + 1]
            )
            for b in range(2):
                y0 = max(16 * b, -dy if dy < 0 else 0)
                y1 = min(16 * b + 16, HH - (dy if dy > 0 else 0))
                if y0 >= y1:
                    continue
                nc.tensor.matmul(
                    pd[:, y0 * WW : y1 * WW],
                    dg[:],
                    ht[:, y0 + dy : y1 + dy, 1 + dx : 1 + dx + WW],
                    start=(n == 0),
                    stop=(n == ntaps - 1),
                )
        if hc == 0:
            dump("HT0", ht[:])
        if gelu_mode == "hw":
            nc.scalar.activation(
                G_sb[hc][:], pd[:], AF.Gelu, bias=dwbpp[:, hc : hc + 1]
            )
        else:
            # sim-only fallback: gelu(x) ~= x * sigmoid(1.702 x)
            hb = pool_ht.tile([P, NTOK], f32, tag="hb", name="hb")
            nc.scalar.activation(
                hb[:], pd[:], AF.Identity, bias=dwbpp[:, hc : hc + 1]
            )
            sg = pool_ht.tile([P, NTOK], f32, tag="sg", name="sg")
            nc.scalar.activation(sg[:], hb[:], AF.Sigmoid, scale=1.702)
            nc.vector.tensor_mul(G_sb[hc][:], hb[:], sg[:])

    pool_dg.release()
    pool_ht.release()

    # ================= S10: fc2 =================
    pool_d.release()
    pool_f = tc.alloc_tile_pool(name="poolF", bufs=1)
    FT_sb = [pool_f.tile([P, NTOK], f32, tag=f"FT{c}", name=f"FT{c}") for c in range(CT)]
    for oc in range(CT):
        for nqc in range(2):
            ps = pool_pmm.tile([P, 512], f32, tag="mm", name="mm")
            for hc in range(HCT):
                nc.tensor.matmul(
                    ps[:],
                    w2_sb[hc][:, ts(oc, P)],
                    G_sb[hc][:, ts(nqc, 512)],
                    start=(hc == 0),
                    stop=(hc == HCT - 1),
                )
            nc.scalar.activation(
                FT_sb[oc][:, ts(nqc, 512)], ps[:], AF.Identity,
                bias=f2bpp[:, oc : oc + 1],
            )

    dump("G0", G_sb[0][:])
    dump("FT0", FT_sb[0][:])
    pool_e.release()

    # ================= S11: residual 2 + output =================
    for i in range(NT):
        ot = pool_out.tile([P, C], f32, tag="out", name="out")
        for c in range(CT):
            pt = pool_pmm.tile([P, P], f32, tag="mm", name="mm")
            nc.tensor.transpose(pt[:], FT_sb[c][:, ts(i, P)], ident[:])
            nc.vector.tensor_add(ot[:, ts(c, P)], pt[:], x1_sb[i][:, ts(c, P)])
        nc.sync.dma_start(out_ap[ts(i, P), :], ot[:])

    pool_f.release()
    for p in (pool_pbig, pool_pmm, pool_out, pool_stats, pool_x1, pool_x, pool_const):
        p.release()


_SHAPES = {
    "x": (NTOK, C),
    "proj_b": (C,),
    "ln1_g": (C,),
    "ln1_b": (C,),
    "ln2_g": (C,),
    "ln2_b": (C,),
    "fc1_b": (HID,),
    "dw_w9": (HID, 9),
    "dw_b": (HID,),
    "fc2_b": (C,),
}
_BF16_SHAPES = {
    "qkv_wTb": (C, 3 * C),
    "proj_wTb": (C, C),
    "fc1_wTb": (C, HID),
    "fc2_wTb": (HID, C),
}


DBG_SPECS = {
    "xlnT0": ((P, NTOK), "bf16"),
    "Q0": ((P, NTOK), "bf16"),
    "K0": ((P, NTOK), "bf16"),
    "V0": ((P, NH, D + 1), "bf16"),
    "E0": ((P, NTOK), "bf16"),
    "Rh0": ((D, NTOK), "f32"),
    "OT0": ((D, NTOK), "bf16"),
    "PT0": ((P, NTOK), "f32"),
    "x1_0": ((P, C), "f32"),
    "x2T0": ((P, NTOK), "bf16"),
    "HT0": ((P, HH, WW + 2), "bf16"),
    "G0": ((P, NTOK), "bf16"),
    "FT0": ((P, NTOK), "f32"),
}


def build_program(gelu_mode="hw", dbg=False):
    nc = bacc.Bacc(
        "TRN2",
        target_bir_lowering=False,
        debug=False,
        enable_asserts=False,
        num_devices=N_CORES,
    )
    d = {}
    for name, shape in _SHAPES.items():
        d[name] = nc.dram_tensor(name, list(shape), f32, kind="ExternalInput").ap()
    for name, shape in _BF16_SHAPES.items():
        d[name] = nc.dram_tensor(name, list(shape), bf16, kind="ExternalInput").ap()
    out_ap = nc.dram_tensor("out", [NTOK, C], f32, kind="ExternalOutput").ap()
    dbg_aps = None
    if dbg:
        dbg_aps = {}
        for k, (shape, dt_) in DBG_SPECS.items():
            dbg_aps[k] = nc.dram_tensor(
                f"dbg_{k}", list(shape), bf16 if dt_ == "bf16" else f32,
                kind="ExternalOutput",
            ).ap()
    with tile.TileContext(nc) as tc:
        _emit(tc, d, out_ap, gelu_mode=gelu_mode, dbg=dbg_aps)
    nc.compile()
    return nc


_CACHE = {}
LAST_RESULT = None


def prep_core_inputs(x_b, w):
    """Per-core input map: x_b is this core's [1024, 512] batch slice,
    w the shared host-prepped weight dict."""
    m = {"x": np.ascontiguousarray(x_b, dtype=np.float32)}
    m.update(w)
    return m


def prep_weights(inputs):
    qkv_raw = np.asarray(inputs["qkv_w"], np.float32).T  # [C, 3C], head-interleaved
    # reorder columns to [Qpair0..3 | Kpair0..3 | V(head-major)], folding the
    # 1/sqrt(d) score scale into the q columns
    qkv_wT = np.empty((C, 3 * C), np.float32)
    for h in range(NH):
        qcol = qkv_raw[:, h * 3 * D : h * 3 * D + D] * (D ** -0.5)
        kcol = qkv_raw[:, h * 3 * D + D : h * 3 * D + 2 * D]
        vcol = qkv_raw[:, h * 3 * D + 2 * D : h * 3 * D + 3 * D]
        qkv_wT[:, h * D : (h + 1) * D] = qcol
        qkv_wT[:, C + h * D : C + (h + 1) * D] = kcol
        qkv_wT[:, 2 * C + h * D : 2 * C + (h + 1) * D] = vcol
    w = {
        "qkv_wTb": np.ascontiguousarray(qkv_wT).astype(ml_dtypes.bfloat16),
        "proj_wTb": np.ascontiguousarray(
            np.asarray(inputs["proj_w"], np.float32).T
        ).astype(ml_dtypes.bfloat16),
        "proj_b": np.asarray(inputs["proj_b"], np.float32),
        "ln1_g": np.asarray(inputs["ln1_g"], np.float32),
        "ln1_b": np.asarray(inputs["ln1_b"], np.float32),
        "ln2_g": np.asarray(inputs["ln2_g"], np.float32),
        "ln2_b": np.asarray(inputs["ln2_b"], np.float32),
        "fc1_wTb": np.ascontiguousarray(
            np.asarray(inputs["fc1_w"], np.float32).T
        ).astype(ml_dtypes.bfloat16),
        "fc1_b": np.asarray(inputs["fc1_b"], np.float32),
        "dw_w9": np.ascontiguousarray(
            np.asarray(inputs["dw_w"], np.float32).reshape(HID, 9)
        ),
        "dw_b": np.asarray(inputs["dw_b"], np.float32),
        "fc2_wTb": np.ascontiguousarray(
            np.asarray(inputs["fc2_w"], np.float32).T
        ).astype(ml_dtypes.bfloat16),
        "fc2_b": np.asarray(inputs["fc2_b"], np.float32),
    }
    return w


def kernel(**inputs):
    x = np.asarray(inputs["x"], np.float32)  # [8, 1024, 512]
    assert x.shape == (N_CORES, NTOK, C), x.shape
    w = prep_weights(inputs)
    if "nc" not in _CACHE:
        _CACHE["nc"] = build_program()
    nc = _CACHE["nc"]
    in_maps = [prep_core_inputs(x[i], w) for i in range(N_CORES)]
    res = bass_utils.run_bass_kernel_spmd(nc, in_maps, core_ids=list(range(N_CORES)))
    global LAST_RESULT
    LAST_RESULT = res
    out = np.stack([res.results[i]["out"] for i in range(N_CORES)], axis=0)
    return out.astype(np.float32)



# revision 29
# speedup vs baseline: 1.2341x; 1.2341x over previous
"""Trainium2 Bass kernel for a SegFormer-style transformer block.

Reference computation (per batch element b):
    x  = x + attention(LN1(x))          # 8 heads, d=64, no qkv bias
    x  = x + mixffn(LN2(x))             # fc1 -> dwconv3x3 -> gelu -> fc2

Sharding: pure data-parallel over batch B=8 across the 8 NeuronCores
(one batch element per core, weights replicated, no collectives).

Per-core layout strategy (v3):
  - LayerNorm stats run token-major; the transpose to feature-major goes
    through the DMA XBAR (dma_start_transpose, 16x128 tiles, two token
    tiles per transfer) instead of the PE.
  - Attention processes heads in PAIRS: head 2t lives on partitions 0:64,
    head 2t+1 on 64:128.  Score matmuls (K=64) for the two heads run
    CONCURRENTLY in the PE array via row tile_position (0,0)/(64,0);
    A@V matmuls (M=64) run concurrently via col tile_position (0,0)/(0,64).
    Softmax denominators come from an all-ones [128,64] stationary matmul
    (output = denominator broadcast across 64 partitions, same PSUM bank
    as A@V's pair so the approximate reciprocal runs at base partition 0).
  - The attention is software-pipelined: the exp evacuations pace the
    phase, so pair t's scores interleave with pair t-1's A@V/denominator
    matmuls (pair 0 interleaves with the V projection); E tiles are
    double-buffered across pairs.
  - proj contracts the full 128-dim head pair in one accumulation chain.
  - The depthwise 3x3 conv runs on the PE as 9 diagonal-matrix matmuls
    accumulating in PSUM; diag tiles are built on the DVE in bf16 (4x).
  - PSUM budget (8 banks): pbig 2x[P,1024] + pav 4x[P,512]; matmul
    chains pair both query chunks into one pbig tile so every ACT/DVE
    evacuation is a single 1024-wide instruction (ACT has ~352 cycles of
    fixed overhead per instruction).
  - All matmuls in bf16 (fp32 PSUM accumulation).

Self-contained: hardcodes all shapes; takes full inputs, returns full
output.
"""

import numpy as np
import ml_dtypes

import concourse.bass as bass
import concourse.tile as tile
from concourse import bacc, mybir
from concourse import bass_utils
from concourse.bass import ts, ds
from concourse.masks import make_identity

P = 128
NTOK = 1024
C = 512
HID = 2048
NH = 8
NP = NH // 2        # head pairs
D = 64
HH = 32
WW = 32
NT = NTOK // P      # 8 token tiles
CT = C // P         # 4 feature tiles
HCT = HID // P      # 16 hidden tiles
EPS = 1e-5
N_CORES = 8

f32 = mybir.dt.float32
bf16 = mybir.dt.bfloat16
AF = mybir.ActivationFunctionType
OP = mybir.AluOpType

# tap order: center first so the first matmul in each PSUM accumulation
# group covers the full bank region (sets has_written everywhere).
TAP_ORDER = [(1, 1), (0, 0), (0, 1), (0, 2), (1, 0), (1, 2), (2, 0), (2, 1), (2, 2)]


def _emit(tc, d, out_ap, gelu_mode="hw", dbg=None):
    def dump(key, ap):
        if dbg is not None and key in dbg:
            tc.nc.sync.dma_start(dbg[key], ap)

    nc = tc.nc

    # ---- whole-kernel pools ----
    pool_const = tc.alloc_tile_pool(name="const", bufs=1)
    pool_x = tc.alloc_tile_pool(name="x", bufs=1)
    pool_x1 = tc.alloc_tile_pool(name="x1", bufs=1)
    pool_stats = tc.alloc_tile_pool(name="stats", bufs=4)
    pool_tt = tc.alloc_tile_pool(name="tt", bufs=3)
    pool_out = tc.alloc_tile_pool(name="outp", bufs=3)
    # PSUM budget (8 banks): pbig 2x[P,1024] + pav (av,dn) x2 [P,512]
    pool_pbig = tc.alloc_tile_pool(name="pbig", bufs=2, space="PSUM")
    pool_pav = tc.alloc_tile_pool(name="pav", bufs=2, space="PSUM")

    identb = pool_const.tile([P, P], bf16, tag="identb", name="identb")
    make_identity(nc, identb[:])
    ones64 = pool_const.tile([P, D], bf16, tag="ones64", name="ones64")
    nc.vector.memset(ones64[:], 1.0)
    zconst = pool_const.tile([P, 1], f32, tag="zconst", name="zconst")
    nc.vector.memset(zconst[:], 0.0)
    nc.const_aps.aps[(f32, 0.0)] = zconst[:]
    epsap = pool_const.tile([P, 1], f32, tag="epsap", name="epsap")
    nc.vector.memset(epsap[:], EPS)

    def pp_load(name, cols, tag):
        t = pool_const.tile([P, cols], f32, tag=tag)
        nc.sync.dma_start(t[:], d[name].rearrange("(o p) -> p o", p=P))
        return t

    g1pp = pp_load("ln1_g", CT, "g1")
    b1pp = pp_load("ln1_b", CT, "b1")
    g2pp = pp_load("ln2_g", CT, "g2")
    b2pp = pp_load("ln2_b", CT, "b2")
    pbpp = pp_load("proj_b", CT, "pb")
    f1bpp = pp_load("fc1_b", HCT, "f1b")
    dwbpp = pp_load("dw_b", HCT, "dwb")
    f2bpp = pp_load("fc2_b", CT, "f2b")
    dwpp = pool_const.tile([P, HCT, 9], f32, tag="dww", name="dww")
    nc.sync.dma_start(dwpp[:], d["dw_w9"].rearrange("(t p) k -> p t k", p=P))

    x_sb = []
    for i in range(NT):
        t = pool_x.tile([P, C], f32, tag=f"x{i}", name=f"x{i}")
        nc.sync.dma_start(t[:], d["x"][ts(i, P), :])
        x_sb.append(t)
    x1_sb = [pool_x1.tile([P, C], f32, tag=f"x1_{i}", name=f"x1_{i}") for i in range(NT)]

    def emit_ln(src_tiles, gpp, bpp, dstT, pool_xn):
        """Token-major LN over C; transpose to feature-major via DMA XBAR
        (two token tiles batched per transpose).

        dstT is a single [P, CT, NTOK] bf16 tile: chunk c holds features
        c*128+p on partitions, tokens on the free axis."""
        for ip in range(NT // 2):
            xn2 = pool_xn.tile([P, 2, C], bf16, tag="xn", name="xn")
            for i2 in range(2):
                i = 2 * ip + i2
                st6 = pool_stats.tile([P, 6], f32, tag="st6", name="st6")
                nc.vector.bn_stats(st6[:], src_tiles[i][:])
                mv = pool_stats.tile([P, 2], f32, tag="mv", name="mv")
                nc.vector.bn_aggr(mv[:], st6[:])
                sd = pool_stats.tile([P, 1], f32, tag="sd", name="sd")
                nc.scalar.activation(sd[:], mv[:, 1:2], AF.Sqrt, bias=epsap[:, 0:1])
                rstd = pool_stats.tile([P, 1], f32, tag="rstd", name="rstd")
                nc.vector.reciprocal(rstd[:], sd[:])
                nb = pool_stats.tile([P, 1], f32, tag="nb", name="nb")
                nc.vector.scalar_tensor_tensor(
                    nb[:], mv[:, 0:1], -1.0, rstd[:], OP.mult, OP.mult
                )
                nc.scalar.activation(
                    xn2[:, i2, :], src_tiles[i][:], AF.Identity,
                    bias=nb[:], scale=rstd[:],
                )
            # transpose both tiles at once: out chunk index = i2*CT + c
            xr = pool_tt.tile([P, 2, CT, P], bf16, tag="xr", name="xr")
            eng = nc.sync if ip % 2 == 0 else nc.scalar
            eng.dma_start_transpose(xr[:], xn2[:])
            for c in range(CT):
                nc.vector.tensor_scalar(
                    dstT[:, c, ts(ip, 2 * P)].rearrange("p (a b) -> p a b", a=2),
                    xr[:, :, c, :],
                    gpp[:, c : c + 1],
                    bpp[:, c : c + 1],
                    OP.mult,
                    OP.add,
                )

    # ================= LN1 + QKV =================
    pool_a = tc.alloc_tile_pool(name="poolA", bufs=2)
    wq_sb = []
    for ci in range(CT):
        t = pool_a.tile([P, 3 * C], bf16, tag=f"wq{ci}", name=f"wq{ci}")
        nc.sync.dma_start(t[:], d["qkv_wTb"][ts(ci, P), :])
        wq_sb.append(t)
    xlnT = pool_a.tile([P, CT, NTOK], bf16, tag="xlnT", name="xlnT")

    emit_ln(x_sb, g1pp, b1pp, xlnT, pool_a)
    dump("xlnT", xlnT[:])

    pool_b = tc.alloc_tile_pool(name="poolB", bufs=1, side="right")
    Q_sb = [pool_b.tile([P, NTOK], bf16, tag=f"Q{t}", name=f"Q{t}") for t in range(NP)]
    K_sb = [pool_b.tile([P, NTOK], bf16, tag=f"K{t}", name=f"K{t}") for t in range(NP)]
    V_sb = [pool_b.tile([P, NH, D], bf16, tag=f"V{j}", name=f"V{j}") for j in range(NT)]
    # E tiles double-buffered across head pairs (software pipeline)
    EA_sb = [[pool_b.tile([P, NTOK], bf16, tag=f"EA{p}{j}", name=f"EA{p}{j}")
              for j in range(NT)] for p in range(2)]
    EB_sb = [[pool_b.tile([P, NTOK], bf16, tag=f"EB{p}{j}", name=f"EB{p}{j}")
              for j in range(NT)] for p in range(2)]

    # Q/K head-pair chunks (qkv_wT is host-reordered: [Qpairs | Kpairs | V]):
    # psum rows 0-63 = head 2t, 64-127 = head 2t+1.
    for t in range(NP):
        for dst, base in ((Q_sb, 0), (K_sb, C)):
            pq = pool_pbig.tile([P, NTOK], f32, tag="big", name="big")
            for nqc in range(2):
                for ci in range(CT):
                    nc.tensor.matmul(
                        pq[:, ts(nqc, 512)],
                        wq_sb[ci][:, base + t * P : base + (t + 1) * P],
                        xlnT[:, ci, ts(nqc, 512)],
                        start=(ci == 0),
                        stop=(ci == CT - 1),
                    )
            nc.vector.tensor_copy(dst[t][:], pq[:])

    def emit_V(j):
        # V in token-major: [tok, (h, dv)]
        ps = pool_pav.tile([P, 512], f32, tag="av", name="av")
        for ci in range(CT):
            nc.tensor.matmul(
                ps[:],
                xlnT[:, ci, ts(j, P)],
                wq_sb[ci][:, 2 * C : 3 * C],
                start=(ci == 0),
                stop=(ci == CT - 1),
            )
        nc.vector.tensor_copy(
            V_sb[j][:], ps[:].rearrange("p (h r) -> p h r", h=NH)
        )

    # ================= attention (software-pipelined head pairs) ==========
    pend = {}

    def avdn_j(t, j):
        """A@V + ones-denominator matmuls for pair t, key tile j (both
        query chunks).  Heads share banks: A rows 0:64, B rows 64:128."""
        if t not in pend:
            pend[t] = (
                [pool_pav.tile([P, 512], f32, tag="av", name="av") for _ in range(2)],
                [pool_pav.tile([P, 512], f32, tag="dn", name="dn") for _ in range(2)],
            )
        avs, dns = pend[t]
        st, sp = (j == 0), (j == NT - 1)
        par = t % 2
        for nqc in range(2):
            ea = EA_sb[par][j][:, ts(nqc, 512)]
            eb = EB_sb[par][j][:, ts(nqc, 512)]
            av, dn = avs[nqc], dns[nqc]
            nc.tensor.matmul(av[0:D, :], V_sb[j][:, 2 * t, :], ea, start=st, stop=sp,
                             skip_group_check=True)
            nc.tensor.matmul(av[D : 2 * D, :], V_sb[j][:, 2 * t + 1, :], eb,
                             start=st, stop=sp, skip_group_check=True)
            nc.tensor.matmul(dn[0:D, :], ones64[:], ea, start=st, stop=sp,
                             skip_group_check=True)
            nc.tensor.matmul(dn[D : 2 * D, :], ones64[:], eb, start=st, stop=sp,
                             skip_group_check=True)

    def finalize(t):
        """Normalize pair t: OT = av / dn (approx-reciprocal + multiply)."""
        avs, dns = pend.pop(t)
        for nqc in range(2):
            dsx = pool_c.tile([P, 512], f32, tag=f"ds{nqc}", name=f"ds{nqc}")
            nc.vector.reciprocal_approx_fast(out=dsx[:], in_=dns[nqc][:])
            nc.vector.tensor_tensor(
                OT_sb[t][:, ts(nqc, 512)], avs[nqc][:], dsx[:], OP.mult
            )
            if t == 0 and nqc == 1:
                dump("dsA0", dsx[:])
        if t == 0:
            dump("EA0", EA_sb[0][0][:])

    def scores_j(t, j):
        # scores: the two heads run concurrently in the PE (row groups
        # 0-1 vs 2-3 via base-partition tile_position); both query
        # chunks land in one 2-bank psum tile so exp runs 1024 wide.
        pbA = pool_pbig.tile([P, NTOK], f32, tag="big", name="big")
        pbB = pool_pbig.tile([P, NTOK], f32, tag="big", name="big")
        for nq in range(2):
            nc.tensor.matmul(
                pbA[:, ts(nq, 512)], K_sb[t][0:D, ts(j, P)],
                Q_sb[t][0:D, ts(nq, 512)],
            )
            nc.tensor.matmul(
                pbB[:, ts(nq, 512)], K_sb[t][D : 2 * D, ts(j, P)],
                Q_sb[t][D : 2 * D, ts(nq, 512)],
            )
        par = t % 2
        nc.scalar.activation(EA_sb[par][j][:], pbA[:], AF.Exp)
        nc.scalar.activation(EB_sb[par][j][:], pbB[:], AF.Exp)

    # pair 0: fill the exp-paced gaps with the V projection, then free
    # the LN1/qkv-weight pool before allocating the attention-output pool.
    for j in range(NT):
        scores_j(0, j)
        emit_V(j)
    pool_a.release()

    pool_c = tc.alloc_tile_pool(name="poolC", bufs=1)
    OT_sb = [pool_c.tile([P, NTOK], bf16, tag=f"OT{t}", name=f"OT{t}") for t in range(NP)]
    pw_sb = []
    for t in range(NP):
        w = pool_c.tile([P, C], bf16, tag=f"pw{t}", name=f"pw{t}")
        nc.sync.dma_start(w[:], d["proj_wTb"][ts(t, P), :])
        pw_sb.append(w)
    PT_sb = [pool_c.tile([P, NTOK], bf16, tag=f"PT{c}", name=f"PT{c}") for c in range(CT)]

    for t in range(1, NP):
        for j in range(NT):
            scores_j(t, j)
            avdn_j(t - 1, j)
        finalize(t - 1)
    for j in range(NT):
        avdn_j(NP - 1, j)
    finalize(NP - 1)

    dump("OT0", OT_sb[0][:])
    pool_b.release()

    # ================= proj (full 128-dim pair contraction) =================
    for oc in range(CT):
        pq = pool_pbig.tile([P, NTOK], f32, tag="big", name="big")
        for nqc in range(2):
            for t in range(NP):
                nc.tensor.matmul(
                    pq[:, ts(nqc, 512)],
                    pw_sb[t][:, ts(oc, P)],
                    OT_sb[t][:, ts(nqc, 512)],
                    start=(t == 0),
                    stop=(t == NP - 1),
                )
        nc.scalar.activation(
            PT_sb[oc][:], pq[:], AF.Identity, bias=pbpp[:, oc : oc + 1]
        )

    dump("PT0", PT_sb[0][:])
    # ================= residual 1 (DMA XBAR transpose back) =================
    for c in range(CT):
        tt = pool_tt.tile([P, NT, P], bf16, tag="ttr", name="ttr")
        eng = nc.sync if c % 2 == 0 else nc.scalar
        eng.dma_start_transpose(tt[:], PT_sb[c][:])
        for i in range(NT):
            nc.vector.tensor_tensor(
                x1_sb[i][:, ts(c, P)], tt[:, i, :], x_sb[i][:, ts(c, P)], OP.add
            )

    pool_c.release()

    # ================= LN2 =================
    pool_d = tc.alloc_tile_pool(name="poolD", bufs=2)
    x2T = pool_d.tile([P, CT, NTOK], bf16, tag="x2T", name="x2T")
    w1_sb = []
    for ci in range(CT):
        t = pool_d.tile([P, HID], bf16, tag=f"w1{ci}", name=f"w1{ci}")
        nc.sync.dma_start(t[:], d["fc1_wTb"][ts(ci, P), :])
        w1_sb.append(t)
    dump("x1_0", x1_sb[0][:])
    emit_ln(x1_sb, g2pp, b2pp, x2T, pool_d)
    dump("x2T", x2T[:])

    # ================= fc1 + dwconv + gelu (fused per tile) ====
    pool_e = tc.alloc_tile_pool(name="poolE", bufs=1, side="right")
    G_sb = [pool_e.tile([P, NTOK], bf16, tag=f"G{hc}", name=f"G{hc}") for hc in range(HCT)]
    w2_sb = []
    for hc in range(HCT):
        t = pool_e.tile([P, C], bf16, tag=f"w2{hc}", name=f"w2{hc}")
        nc.sync.dma_start(t[:], d["fc2_wTb"][ts(hc, P), :])
        w2_sb.append(t)
    pool_ht = tc.alloc_tile_pool(name="poolHT", bufs=3, side="right")
    pool_dg = tc.alloc_tile_pool(name="poolDG", bufs=4, side="right")

    for hc in range(HCT):
        # HT is x-zero-padded: [P, 32, 34] with zero columns at x=0 and 33,
        # so every conv tap reads a flat contiguous range.
        ht = pool_ht.tile([P, HH, WW + 2], bf16, tag="HT", name="HT")
        nc.vector.memset(ht[:, :, 0:1], 0.0)
        nc.vector.memset(ht[:, :, WW + 1 : WW + 2], 0.0)
        pq = pool_pbig.tile([P, NTOK], f32, tag="big", name="big")
        for nqc in range(2):
            for ci in range(CT):
                nc.tensor.matmul(
                    pq[:, ts(nqc, 512)],
                    w1_sb[ci][:, ts(hc, P)],
                    x2T[:, ci, ts(nqc, 512)],
                    start=(ci == 0),
                    stop=(ci == CT - 1),
                )
        nc.scalar.activation(
            ht[:, :, 1 : WW + 1],
            pq[:].rearrange("p (y x) -> p y x", x=WW),
            AF.Identity,
            bias=f1bpp[:, hc : hc + 1],
        )
        # depthwise 3x3 conv: 9 diagonal matmuls accumulating in PSUM; the
        # diag tiles are built on the DVE (bf16, 4x mode).
        pd = pool_pbig.tile([P, NTOK], f32, tag="big", name="big")
        ntaps = len(TAP_ORDER)
        for n, (ky, kx) in enumerate(TAP_ORDER):
            dy, dx = ky - 1, kx - 1
            dg = pool_dg.tile([P, P], bf16, tag="dg", name="dg")
            nc.vector.tensor_scalar_mul(
                dg[:], identb[:], dwpp[:, hc, 3 * ky + kx : 3 * ky + kx + 1]
            )
            for b in range(2):
                y0 = max(16 * b, -dy if dy < 0 else 0)
                y1 = min(16 * b + 16, HH - (dy if dy > 0 else 0))
                if y0 >= y1:
                    continue
                nc.tensor.matmul(
                    pd[:, y0 * WW : y1 * WW],
                    dg[:],
                    ht[:, y0 + dy : y1 + dy, 1 + dx : 1 + dx + WW],
                    start=(n == 0),
                    stop=(n == ntaps - 1),
                )
        if hc == 0:
            dump("HT0", ht[:])
        if gelu_mode == "hw":
            nc.scalar.activation(
                G_sb[hc][:], pd[:], AF.Gelu, bias=dwbpp[:, hc : hc + 1]
            )
            if hc == 0:
                dump("G0", G_sb[0][:])
        else:
            # sim-only fallback: gelu(x) ~= x * sigmoid(1.702 x)
            hb = pool_ht.tile([P, NTOK], f32, tag="hb", name="hb")
            nc.scalar.activation(
                hb[:], pd[:], AF.Identity, bias=dwbpp[:, hc : hc + 1]
            )
            sg = pool_ht.tile([P, NTOK], f32, tag="sg", name="sg")
            nc.scalar.activation(sg[:], hb[:], AF.Sigmoid, scale=1.702)
            nc.vector.tensor_mul(G_sb[hc][:], hb[:], sg[:])

    pool_dg.release()
    pool_ht.release()

    # ================= fc2 =================
    pool_d.release()
    pool_f = tc.alloc_tile_pool(name="poolF", bufs=1)
    FT_sb = [pool_f.tile([P, NTOK], bf16, tag=f"FT{c}", name=f"FT{c}") for c in range(CT)]
    for oc in range(CT):
        pq = pool_pbig.tile([P, NTOK], f32, tag="big", name="big")
        for nqc in range(2):
            for hc in range(HCT):
                nc.tensor.matmul(
                    pq[:, ts(nqc, 512)],
                    w2_sb[hc][:, ts(oc, P)],
                    G_sb[hc][:, ts(nqc, 512)],
                    start=(hc == 0),
                    stop=(hc == HCT - 1),
                )
        nc.scalar.activation(
            FT_sb[oc][:], pq[:], AF.Identity, bias=f2bpp[:, oc : oc + 1]
        )

    dump("FT0", FT_sb[0][:])
    pool_e.release()

    # ================= residual 2 + output =================
    ttF = []
    for c in range(CT):
        tt = pool_f.tile([P, NT, P], bf16, tag=f"ttf{c}", name=f"ttf{c}")
        eng = nc.sync if c % 2 == 0 else nc.scalar
        eng.dma_start_transpose(tt[:], FT_sb[c][:])
        ttF.append(tt)
    for i in range(NT):
        ot = pool_out.tile([P, C], f32, tag="out", name="out")
        for c in range(CT):
            nc.vector.tensor_tensor(
                ot[:, ts(c, P)], ttF[c][:, i, :], x1_sb[i][:, ts(c, P)], OP.add
            )
        nc.sync.dma_start(out_ap[ts(i, P), :], ot[:])

    pool_f.release()
    for p in (pool_pav, pool_pbig, pool_out, pool_tt, pool_stats, pool_x1, pool_x, pool_const):
        p.release()


_SHAPES = {
    "x": (NTOK, C),
    "proj_b": (C,),
    "ln1_g": (C,),
    "ln1_b": (C,),
    "ln2_g": (C,),
    "ln2_b": (C,),
    "fc1_b": (HID,),
    "dw_w9": (HID, 9),
    "dw_b": (HID,),
    "fc2_b": (C,),
}
_BF16_SHAPES = {
    "qkv_wTb": (C, 3 * C),
    "proj_wTb": (C, C),
    "fc1_wTb": (C, HID),
    "fc2_wTb": (HID, C),
}


DBG_SPECS = {
    "xlnT": ((P, CT, NTOK), "bf16"),
    "EA0": ((P, NTOK), "bf16"),
    "dsA0": ((P, 512), "f32"),
    "OT0": ((P, NTOK), "bf16"),
    "PT0": ((P, NTOK), "bf16"),
    "x1_0": ((P, C), "f32"),
    "x2T": ((P, CT, NTOK), "bf16"),
    "HT0": ((P, HH, WW + 2), "bf16"),
    "G0": ((P, NTOK), "bf16"),
    "FT0": ((P, NTOK), "bf16"),
}


def build_program(gelu_mode="hw", dbg=False):
    nc = bacc.Bacc(
        "TRN2",
        target_bir_lowering=False,
        debug=False,
        enable_asserts=False,
        num_devices=N_CORES,
    )
    d = {}
    for name, shape in _SHAPES.items():
        d[name] = nc.dram_tensor(name, list(shape), f32, kind="ExternalInput").ap()
    for name, shape in _BF16_SHAPES.items():
        d[name] = nc.dram_tensor(name, list(shape), bf16, kind="ExternalInput").ap()
    out_ap = nc.dram_tensor("out", [NTOK, C], f32, kind="ExternalOutput").ap()
    dbg_aps = None
    if dbg:
        dbg_aps = {}
        for k, (shape, dt_) in DBG_SPECS.items():
            dbg_aps[k] = nc.dram_tensor(
                f"dbg_{k}", list(shape), bf16 if dt_ == "bf16" else f32,
                kind="ExternalOutput",
            ).ap()
    with tile.TileContext(nc) as tc:
        _emit(tc, d, out_ap, gelu_mode=gelu_mode, dbg=dbg_aps)
    nc.compile()
    return nc


_CACHE = {}
LAST_RESULT = None


def prep_core_inputs(x_b, w):
    """Per-core input map: x_b is this core's [1024, 512] batch slice,
    w the shared host-prepped weight dict."""
    m = {"x": np.ascontiguousarray(x_b, dtype=np.float32)}
    m.update(w)
    return m


def prep_weights(inputs):
    qkv_raw = np.asarray(inputs["qkv_w"], np.float32).T  # [C, 3C], head-interleaved
    # reorder columns to [Qpair0..3 | Kpair0..3 | V(head-major)], folding the
    # 1/sqrt(d) score scale into the q columns
    qkv_wT = np.empty((C, 3 * C), np.float32)
    for h in range(NH):
        qcol = qkv_raw[:, h * 3 * D : h * 3 * D + D] * (D ** -0.5)
        kcol = qkv_raw[:, h * 3 * D + D : h * 3 * D + 2 * D]
        vcol = qkv_raw[:, h * 3 * D + 2 * D : h * 3 * D + 3 * D]
        qkv_wT[:, h * D : (h + 1) * D] = qcol
        qkv_wT[:, C + h * D : C + (h + 1) * D] = kcol
        qkv_wT[:, 2 * C + h * D : 2 * C + (h + 1) * D] = vcol
    w = {
        "qkv_wTb": np.ascontiguousarray(qkv_wT).astype(ml_dtypes.bfloat16),
        "proj_wTb": np.ascontiguousarray(
            np.asarray(inputs["proj_w"], np.float32).T
        ).astype(ml_dtypes.bfloat16),
        "proj_b": np.asarray(inputs["proj_b"], np.float32),
        "ln1_g": np.asarray(inputs["ln1_g"], np.float32),
        "ln1_b": np.asarray(inputs["ln1_b"], np.float32),
        "ln2_g": np.asarray(inputs["ln2_g"], np.float32),
        "ln2_b": np.asarray(inputs["ln2_b"], np.float32),
        "fc1_wTb": np.ascontiguousarray(
            np.asarray(inputs["fc1_w"], np.float32).T
        ).astype(ml_dtypes.bfloat16),
        "fc1_b": np.asarray(inputs["fc1_b"], np.float32),
        "dw_w9": np.ascontiguousarray(
            np.asarray(inputs["dw_w"], np.float32).reshape(HID, 9)
        ),
        "dw_b": np.asarray(inputs["dw_b"], np.float32),
        "fc2_wTb": np.ascontiguousarray(
            np.asarray(inputs["fc2_w"], np.float32).T
        ).astype(ml_dtypes.bfloat16),
        "fc2_b": np.asarray(inputs["fc2_b"], np.float32),
    }
    return w


def kernel(**inputs):
    x = np.asarray(inputs["x"], np.float32)  # [8, 1024, 512]
    assert x.shape == (N_CORES, NTOK, C), x.shape
    w = prep_weights(inputs)
    if "nc" not in _CACHE:
        _CACHE["nc"] = build_program()
    nc = _CACHE["nc"]
    in_maps = [prep_core_inputs(x[i], w) for i in range(N_CORES)]
    res = bass_utils.run_bass_kernel_spmd(nc, in_maps, core_ids=list(range(N_CORES)))
    global LAST_RESULT
    LAST_RESULT = res
    out = np.stack([res.results[i]["out"] for i in range(N_CORES)], axis=0)
    return out.astype(np.float32)


# revision 36
# speedup vs baseline: 1.3518x; 1.0954x over previous
"""Trainium2 Bass kernel for a SegFormer-style transformer block.

Reference computation (per batch element b):
    x  = x + attention(LN1(x))          # 8 heads, d=64, no qkv bias
    x  = x + mixffn(LN2(x))             # fc1 -> dwconv3x3 -> gelu -> fc2

Sharding: pure data-parallel over batch B=8 across the 8 NeuronCores
(one batch element per core, weights replicated, no collectives).

Per-core layout strategy (v3):
  - LayerNorm stats run token-major; the transpose to feature-major goes
    through the DMA XBAR (dma_start_transpose, 16x128 tiles, two token
    tiles per transfer) instead of the PE.
  - Attention processes heads in PAIRS: head 2t lives on partitions 0:64,
    head 2t+1 on 64:128.  Score matmuls (K=64) for the two heads run
    CONCURRENTLY in the PE array via row tile_position (0,0)/(64,0);
    A@V matmuls (M=64) run concurrently via col tile_position (0,0)/(0,64).
    Softmax denominators come from an all-ones [128,64] stationary matmul
    (output = denominator broadcast across 64 partitions, same PSUM bank
    as A@V's pair so the approximate reciprocal runs at base partition 0).
  - The attention is software-pipelined: the exp evacuations pace the
    phase, so pair t's scores interleave with pair t-1's A@V/denominator
    matmuls (pair 0 interleaves with the V projection); E tiles are
    double-buffered across pairs.
  - proj contracts the full 128-dim head pair in one accumulation chain.
  - The depthwise 3x3 conv runs on the PE as 9 diagonal-matrix matmuls
    accumulating in PSUM; diag tiles are built on the DVE in bf16 (4x).
  - PSUM budget (8 banks): pbig 2x[P,1024] + pav 4x[P,512]; matmul
    chains pair both query chunks into one pbig tile so every ACT/DVE
    evacuation is a single 1024-wide instruction (ACT has ~352 cycles of
    fixed overhead per instruction).
  - All matmuls in bf16 (fp32 PSUM accumulation).

Self-contained: hardcodes all shapes; takes full inputs, returns full
output.
"""

import numpy as np
import ml_dtypes

import concourse.bass as bass
import concourse.tile as tile
from concourse import bacc, mybir
from concourse import bass_utils
from concourse.bass import ts, ds
from concourse.masks import make_identity

P = 128
NTOK = 1024
C = 512
HID = 2048
NH = 8
NP = NH // 2        # head pairs
D = 64
HH = 32
WW = 32
NT = NTOK // P      # 8 token tiles
CT = C // P         # 4 feature tiles
HCT = HID // P      # 16 hidden tiles
EPS = 1e-5
N_CORES = 8

f32 = mybir.dt.float32
bf16 = mybir.dt.bfloat16
AF = mybir.ActivationFunctionType
OP = mybir.AluOpType

# tap order: center first so the first matmul in each PSUM accumulation
# group covers the full bank region (sets has_written everywhere).
TAP_ORDER = [(1, 1), (0, 0), (0, 1), (0, 2), (1, 0), (1, 2), (2, 0), (2, 1), (2, 2)]


def _emit(tc, d, out_ap, gelu_mode="hw", dbg=None):
    def dump(key, ap):
        if dbg is not None and key in dbg:
            tc.nc.sync.dma_start(dbg[key], ap)

    nc = tc.nc

    # ---- whole-kernel pools ----
    pool_const = tc.alloc_tile_pool(name="const", bufs=1)
    pool_x = tc.alloc_tile_pool(name="x", bufs=1)
    pool_x1 = tc.alloc_tile_pool(name="x1", bufs=1)
    pool_stats = tc.alloc_tile_pool(name="stats", bufs=4)
    pool_tt = tc.alloc_tile_pool(name="tt", bufs=3)
    pool_out = tc.alloc_tile_pool(name="outp", bufs=1)
    # PSUM budget (8 banks): pbig 2x[P,1024] + pav (av,dn) x2 [P,512]
    pool_pbig = tc.alloc_tile_pool(name="pbig", bufs=2, space="PSUM")
    pool_pav = tc.alloc_tile_pool(name="pav", bufs=2, space="PSUM")

    identb = pool_const.tile([P, P], bf16, tag="identb", name="identb")
    make_identity(nc, identb[:])
    ones64 = pool_const.tile([P, D], bf16, tag="ones64", name="ones64")
    nc.vector.memset(ones64[:], 1.0)
    zconst = pool_const.tile([P, 1], f32, tag="zconst", name="zconst")
    nc.vector.memset(zconst[:], 0.0)
    nc.const_aps.aps[(f32, 0.0)] = zconst[:]
    epsap = pool_const.tile([P, 1], f32, tag="epsap", name="epsap")
    nc.vector.memset(epsap[:], EPS)

    def pp_load(name, cols, tag):
        t = pool_const.tile([P, cols], f32, tag=tag)
        nc.sync.dma_start(t[:], d[name].rearrange("(o p) -> p o", p=P))
        return t

    g1pp = pp_load("ln1_g", CT, "g1")
    b1pp = pp_load("ln1_b", CT, "b1")
    g2pp = pp_load("ln2_g", CT, "g2")
    b2pp = pp_load("ln2_b", CT, "b2")
    pbpp = pp_load("proj_b", CT, "pb")
    f1bpp = pp_load("fc1_b", HCT, "f1b")
    dwbpp = pp_load("dw_b", HCT, "dwb")
    f2bpp = pp_load("fc2_b", CT, "f2b")
    dwpp = pool_const.tile([P, HCT, 9], f32, tag="dww", name="dww")
    nc.sync.dma_start(dwpp[:], d["dw_w9"].rearrange("(t p) k -> p t k", p=P))

    x_sb = []
    for i in range(NT):
        t = pool_x.tile([P, C], f32, tag=f"x{i}", name=f"x{i}")
        nc.sync.dma_start(t[:], d["x"][ts(i, P), :])
        x_sb.append(t)
    x1_sb = [pool_x1.tile([P, C], f32, tag=f"x1_{i}", name=f"x1_{i}") for i in range(NT)]

    def emit_ln(src_tiles, gpp, bpp, dstT, pool_xn, pairs=None, prestats=None):
        """Token-major LN over C; transpose to feature-major via DMA XBAR
        (two token tiles batched per transpose).

        dstT is a single [P, CT, NTOK] bf16 tile: chunk c holds features
        c*128+p on partitions, tokens on the free axis.  prestats, if
        given, maps tile index -> [P, CT, 6] per-chunk bn_stats tile
        (computed earlier, e.g. fused into the residual adds)."""
        for ip in pairs if pairs is not None else range(NT // 2):
            xn2 = pool_xn.tile([P, 2, C], bf16, tag="xn", name="xn")
            for i2 in range(2):
                i = 2 * ip + i2
                if prestats is None:
                    st6 = pool_stats.tile([P, 6], f32, tag="st6", name="st6")
                    nc.vector.bn_stats(st6[:], src_tiles[i][:])
                    st6_ap = st6[:]
                else:
                    st6_ap = prestats[i][:]
                mv = pool_stats.tile([P, 2], f32, tag="mv", name="mv")
                nc.vector.bn_aggr(mv[:], st6_ap)
                sd = pool_stats.tile([P, 1], f32, tag="sd", name="sd")
                nc.scalar.activation(sd[:], mv[:, 1:2], AF.Sqrt, bias=epsap[:, 0:1])
                rstd = pool_stats.tile([P, 1], f32, tag="rstd", name="rstd")
                nc.vector.reciprocal(rstd[:], sd[:])
                nb = pool_stats.tile([P, 1], f32, tag="nb", name="nb")
                nc.vector.scalar_tensor_tensor(
                    nb[:], mv[:, 0:1], -1.0, rstd[:], OP.mult, OP.mult
                )
                nc.scalar.activation(
                    xn2[:, i2, :], src_tiles[i][:], AF.Identity,
                    bias=nb[:], scale=rstd[:],
                )
            # transpose both tiles at once: out chunk index = i2*CT + c
            xr = pool_tt.tile([P, 2, CT, P], bf16, tag="xr", name="xr")
            eng = nc.sync if ip % 2 == 0 else nc.scalar
            eng.dma_start_transpose(xr[:], xn2[:])
            for c in range(CT):
                nc.vector.tensor_scalar(
                    dstT[:, c, ts(ip, 2 * P)].rearrange("p (a b) -> p a b", a=2),
                    xr[:, :, c, :],
                    gpp[:, c : c + 1],
                    bpp[:, c : c + 1],
                    OP.mult,
                    OP.add,
                )

    # ================= LN1 + QKV =================
    pool_a = tc.alloc_tile_pool(name="poolA", bufs=2)
    wq_sb = []
    for ci in range(CT):
        t = pool_a.tile([P, 3 * C], bf16, tag=f"wq{ci}", name=f"wq{ci}")
        nc.sync.dma_start(t[:], d["qkv_wTb"][ts(ci, P), :])
        wq_sb.append(t)
    xlnT = pool_a.tile([P, CT, NTOK], bf16, tag="xlnT", name="xlnT")

    pool_b = tc.alloc_tile_pool(name="poolB", bufs=1, side="right")
    Q_sb = [pool_b.tile([P, NTOK], bf16, tag=f"Q{t}", name=f"Q{t}") for t in range(NP)]
    K_sb = [pool_b.tile([P, NTOK], bf16, tag=f"K{t}", name=f"K{t}") for t in range(NP)]
    V_sb = [pool_b.tile([P, NH, D], bf16, tag=f"V{j}", name=f"V{j}") for j in range(NT)]
    # E tiles double-buffered across head pairs (software pipeline)
    EA_sb = [[pool_b.tile([P, NTOK], bf16, tag=f"EA{p}{j}", name=f"EA{p}{j}")
              for j in range(NT)] for p in range(2)]
    EB_sb = [[pool_b.tile([P, NTOK], bf16, tag=f"EB{p}{j}", name=f"EB{p}{j}")
              for j in range(NT)] for p in range(2)]

    def emit_qk(nqc):
        # Q/K head-pair chunks (qkv_wT host-reordered [Qpairs|Kpairs|V]):
        # psum rows 0-63 = head 2t, 64-127 = head 2t+1.
        for t in range(NP):
            for dst, base, tg in ((Q_sb, 0, "av"), (K_sb, C, "dn")):
                ps = pool_pav.tile([P, 512], f32, tag=tg, name=tg)
                for ci in range(CT):
                    nc.tensor.matmul(
                        ps[:],
                        wq_sb[ci][:, base + t * P : base + (t + 1) * P],
                        xlnT[:, ci, ts(nqc, 512)],
                        start=(ci == 0),
                        stop=(ci == CT - 1),
                    )
                nc.vector.tensor_copy(dst[t][:, ts(nqc, 512)], ps[:])

    # interleave LN1 halves with Q/K halves so the PE engages early
    emit_ln(x_sb, g1pp, b1pp, xlnT, pool_a, pairs=(0, 1))
    emit_qk(0)
    emit_ln(x_sb, g1pp, b1pp, xlnT, pool_a, pairs=(2, 3))
    emit_qk(1)
    dump("xlnT", xlnT[:])

    def emit_V(j):
        # V in token-major: [tok, (h, dv)]
        ps = pool_pav.tile([P, 512], f32, tag="av", name="av")
        for ci in range(CT):
            nc.tensor.matmul(
                ps[:],
                xlnT[:, ci, ts(j, P)],
                wq_sb[ci][:, 2 * C : 3 * C],
                start=(ci == 0),
                stop=(ci == CT - 1),
            )
        nc.vector.tensor_copy(
            V_sb[j][:], ps[:].rearrange("p (h r) -> p h r", h=NH)
        )

    # ================= attention (software-pipelined head pairs) ==========
    pend = {}

    def avdn_j(t, j):
        """A@V + ones-denominator matmuls for pair t, key tile j (both
        query chunks).  Heads share banks: A rows 0:64, B rows 64:128."""
        if t not in pend:
            pend[t] = (
                [pool_pav.tile([P, 512], f32, tag="av", name="av") for _ in range(2)],
                [pool_pav.tile([P, 512], f32, tag="dn", name="dn") for _ in range(2)],
            )
        avs, dns = pend[t]
        st, sp = (j == 0), (j == NT - 1)
        par = t % 2
        for nqc in range(2):
            ea = EA_sb[par][j][:, ts(nqc, 512)]
            eb = EB_sb[par][j][:, ts(nqc, 512)]
            av, dn = avs[nqc], dns[nqc]
            nc.tensor.matmul(av[0:D, :], V_sb[j][:, 2 * t, :], ea, start=st, stop=sp,
                             skip_group_check=True)
            nc.tensor.matmul(av[D : 2 * D, :], V_sb[j][:, 2 * t + 1, :], eb,
                             start=st, stop=sp, skip_group_check=True)
            nc.tensor.matmul(dn[0:D, :], ones64[:], ea, start=st, stop=sp,
                             skip_group_check=True)
            nc.tensor.matmul(dn[D : 2 * D, :], ones64[:], eb, start=st, stop=sp,
                             skip_group_check=True)

    def finalize(t):
        """Normalize pair t: OT = av / dn (approx-reciprocal + multiply)."""
        avs, dns = pend.pop(t)
        for nqc in range(2):
            dsx = pool_c.tile([P, 512], f32, tag=f"ds{nqc}", name=f"ds{nqc}")
            nc.vector.reciprocal_approx_fast(out=dsx[:], in_=dns[nqc][:])
            nc.vector.tensor_tensor(
                OT_sb[t][:, ts(nqc, 512)], avs[nqc][:], dsx[:], OP.mult
            )
            if t == 0 and nqc == 1:
                dump("dsA0", dsx[:])
        if t == 0:
            dump("EA0", EA_sb[0][0][:])

    def scores_j(t, j):
        # scores: the two heads run concurrently in the PE (row groups
        # 0-1 vs 2-3 via base-partition tile_position); both query
        # chunks land in one 2-bank psum tile so exp runs 1024 wide.
        pbA = pool_pbig.tile([P, NTOK], f32, tag="big", name="big")
        pbB = pool_pbig.tile([P, NTOK], f32, tag="big", name="big")
        for nq in range(2):
            nc.tensor.matmul(
                pbA[:, ts(nq, 512)], K_sb[t][0:D, ts(j, P)],
                Q_sb[t][0:D, ts(nq, 512)],
            )
            nc.tensor.matmul(
                pbB[:, ts(nq, 512)], K_sb[t][D : 2 * D, ts(j, P)],
                Q_sb[t][D : 2 * D, ts(nq, 512)],
            )
        par = t % 2
        nc.scalar.activation(EA_sb[par][j][:], pbA[:], AF.Exp)
        nc.scalar.activation(EB_sb[par][j][:], pbB[:], AF.Exp)

    # pair 0: fill the exp-paced gaps with the V projection, then free
    # the LN1/qkv-weight pool before allocating the attention-output pool.
    for j in range(NT):
        scores_j(0, j)
        emit_V(j)
    pool_a.release()

    pool_c = tc.alloc_tile_pool(name="poolC", bufs=1)
    OT_sb = [pool_c.tile([P, NTOK], bf16, tag=f"OT{t}", name=f"OT{t}") for t in range(NP)]
    pw_sb = []
    for t in range(NP):
        w = pool_c.tile([P, C], bf16, tag=f"pw{t}", name=f"pw{t}")
        nc.sync.dma_start(w[:], d["proj_wTb"][ts(t, P), :])
        pw_sb.append(w)
    PT_sb = [pool_c.tile([P, NTOK], bf16, tag=f"PT{c}", name=f"PT{c}") for c in range(CT)]

    for t in range(1, NP):
        for j in range(NT):
            scores_j(t, j)
            avdn_j(t - 1, j)
        finalize(t - 1)
    for j in range(NT):
        avdn_j(NP - 1, j)
    finalize(NP - 1)

    dump("OT0", OT_sb[0][:])
    pool_b.release()

    # ===== proj + residual 1 + per-chunk LN2 stats (fused per c-chunk) =====
    st6_2 = [pool_stats.tile([P, CT, 6], f32, tag=f"st2_{i}", name=f"st2_{i}")
             for i in range(NT)]
    for oc in range(CT):
        pq = pool_pbig.tile([P, NTOK], f32, tag="big", name="big")
        for nqc in range(2):
            for t in range(NP):
                nc.tensor.matmul(
                    pq[:, ts(nqc, 512)],
                    pw_sb[t][:, ts(oc, P)],
                    OT_sb[t][:, ts(nqc, 512)],
                    start=(t == 0),
                    stop=(t == NP - 1),
                )
        nc.scalar.activation(
            PT_sb[oc][:], pq[:], AF.Identity, bias=pbpp[:, oc : oc + 1]
        )
        tt = pool_tt.tile([P, NT, P], bf16, tag="ttr", name="ttr")
        eng = nc.sync if oc % 2 == 0 else nc.scalar
        eng.dma_start_transpose(tt[:], PT_sb[oc][:])
        for i in range(NT):
            nc.vector.tensor_tensor(
                x1_sb[i][:, ts(oc, P)], tt[:, i, :], x_sb[i][:, ts(oc, P)], OP.add
            )
            nc.vector.bn_stats(st6_2[i][:, oc, :], x1_sb[i][:, ts(oc, P)])

    dump("PT0", PT_sb[0][:])
    pool_c.release()

    # ================= LN2 =================
    pool_d = tc.alloc_tile_pool(name="poolD", bufs=2)
    x2T = pool_d.tile([P, CT, NTOK], bf16, tag="x2T", name="x2T")
    w1_sb = []
    for ci in range(CT):
        t = pool_d.tile([P, HID], bf16, tag=f"w1{ci}", name=f"w1{ci}")
        nc.sync.dma_start(t[:], d["fc1_wTb"][ts(ci, P), :])
        w1_sb.append(t)
    dump("x1_0", x1_sb[0][:])
    emit_ln(x1_sb, g2pp, b2pp, x2T, pool_d, prestats=st6_2)
    dump("x2T", x2T[:])

    # ================= fc1 + dwconv + gelu (fused per tile) ====
    pool_e = tc.alloc_tile_pool(name="poolE", bufs=1, side="right")
    G_sb = [pool_e.tile([P, NTOK], bf16, tag=f"G{hc}", name=f"G{hc}") for hc in range(HCT)]
    w2_sb = []
    for hc in range(HCT):
        t = pool_e.tile([P, C], bf16, tag=f"w2{hc}", name=f"w2{hc}")
        nc.sync.dma_start(t[:], d["fc2_wTb"][ts(hc, P), :])
        w2_sb.append(t)
    pool_ht = tc.alloc_tile_pool(name="poolHT", bufs=3, side="right")
    pool_dg = tc.alloc_tile_pool(name="poolDG", bufs=4, side="right")

    for hc in range(HCT):
        # HT is x-zero-padded: [P, 32, 34] with zero columns at x=0 and 33,
        # so every conv tap reads a flat contiguous range.
        ht = pool_ht.tile([P, HH, WW + 2], bf16, tag="HT", name="HT")
        nc.vector.memset(ht[:, :, 0:1], 0.0)
        nc.vector.memset(ht[:, :, WW + 1 : WW + 2], 0.0)
        # fc1 runs in the pav banks so the conv's pd tiles get the pbig
        # ring to themselves (full hc-to-hc overlap).
        for nqc in range(2):
            ps = pool_pav.tile([P, 512], f32, tag=("av", "dn")[nqc], name="f1")
            for ci in range(CT):
                nc.tensor.matmul(
                    ps[:],
                    w1_sb[ci][:, ts(hc, P)],
                    x2T[:, ci, ts(nqc, 512)],
                    start=(ci == 0),
                    stop=(ci == CT - 1),
                )
            nc.scalar.activation(
                ht[:, 16 * nqc : 16 * nqc + 16, 1 : WW + 1],
                ps[:].rearrange("p (y x) -> p y x", x=WW),
                AF.Identity,
                bias=f1bpp[:, hc : hc + 1],
            )
        # depthwise 3x3 conv: 9 diagonal matmuls accumulating in PSUM; the
        # diag tiles are built on the DVE (bf16, 4x mode).
        pd = pool_pbig.tile([P, NTOK], f32, tag="big", name="big")
        ntaps = len(TAP_ORDER)
        for n, (ky, kx) in enumerate(TAP_ORDER):
            dy, dx = ky - 1, kx - 1
            dg = pool_dg.tile([P, P], bf16, tag="dg", name="dg")
            nc.vector.tensor_scalar_mul(
                dg[:], identb[:], dwpp[:, hc, 3 * ky + kx : 3 * ky + kx + 1]
            )
            for b in range(2):
                y0 = max(16 * b, -dy if dy < 0 else 0)
                y1 = min(16 * b + 16, HH - (dy if dy > 0 else 0))
                if y0 >= y1:
                    continue
                nc.tensor.matmul(
                    pd[:, y0 * WW : y1 * WW],
                    dg[:],
                    ht[:, y0 + dy : y1 + dy, 1 + dx : 1 + dx + WW],
                    start=(n == 0),
                    stop=(n == ntaps - 1),
                )
        if hc == 0:
            dump("HT0", ht[:])
        if gelu_mode == "hw":
            nc.scalar.activation(
                G_sb[hc][:], pd[:], AF.Gelu, bias=dwbpp[:, hc : hc + 1]
            )
            if hc == 0:
                dump("G0", G_sb[0][:])
        else:
            # sim-only fallback: gelu(x) ~= x * sigmoid(1.702 x)
            hb = pool_ht.tile([P, NTOK], f32, tag="hb", name="hb")
            nc.scalar.activation(
                hb[:], pd[:], AF.Identity, bias=dwbpp[:, hc : hc + 1]
            )
            sg = pool_ht.tile([P, NTOK], f32, tag="sg", name="sg")
            nc.scalar.activation(sg[:], hb[:], AF.Sigmoid, scale=1.702)
            nc.vector.tensor_mul(G_sb[hc][:], hb[:], sg[:])

    pool_dg.release()
    pool_ht.release()

    # ================= fc2 + residual 2 + output (fused per oc) ===========
    pool_d.release()
    pool_f = tc.alloc_tile_pool(name="poolF", bufs=1)
    FT_sb = [pool_f.tile([P, NTOK], bf16, tag=f"FT{c}", name=f"FT{c}") for c in range(CT)]
    ot_sb = [pool_out.tile([P, C], f32, tag=f"ot{i}", name=f"ot{i}") for i in range(NT)]
    for oc in range(CT):
        pq = pool_pbig.tile([P, NTOK], f32, tag="big", name="big")
        for nqc in range(2):
            for hc in range(HCT):
                nc.tensor.matmul(
                    pq[:, ts(nqc, 512)],
                    w2_sb[hc][:, ts(oc, P)],
                    G_sb[hc][:, ts(nqc, 512)],
                    start=(hc == 0),
                    stop=(hc == HCT - 1),
                )
        nc.scalar.activation(
            FT_sb[oc][:], pq[:], AF.Identity, bias=f2bpp[:, oc : oc + 1]
        )
        tt = pool_f.tile([P, NT, P], bf16, tag=f"ttf{oc}", name=f"ttf{oc}")
        eng = nc.sync if oc % 2 == 0 else nc.scalar
        eng.dma_start_transpose(tt[:], FT_sb[oc][:])
        for i in range(NT):
            nc.vector.tensor_tensor(
                ot_sb[i][:, ts(oc, P)], tt[:, i, :], x1_sb[i][:, ts(oc, P)], OP.add
            )
            if oc == CT - 1:
                nc.sync.dma_start(out_ap[ts(i, P), :], ot_sb[i][:])

    dump("FT0", FT_sb[0][:])
    pool_e.release()
    pool_f.release()
    for p in (pool_pav, pool_pbig, pool_out, pool_tt, pool_stats, pool_x1, pool_x, pool_const):
        p.release()


_SHAPES = {
    "x": (NTOK, C),
    "proj_b": (C,),
    "ln1_g": (C,),
    "ln1_b": (C,),
    "ln2_g": (C,),
    "ln2_b": (C,),
    "fc1_b": (HID,),
    "dw_w9": (HID, 9),
    "dw_b": (HID,),
    "fc2_b": (C,),
}
_BF16_SHAPES = {
    "qkv_wTb": (C, 3 * C),
    "proj_wTb": (C, C),
    "fc1_wTb": (C, HID),
    "fc2_wTb": (HID, C),
}


DBG_SPECS = {
    "xlnT": ((P, CT, NTOK), "bf16"),
    "EA0": ((P, NTOK), "bf16"),
    "dsA0": ((P, 512), "f32"),
    "OT0": ((P, NTOK), "bf16"),
    "PT0": ((P, NTOK), "bf16"),
    "x1_0": ((P, C), "f32"),
    "x2T": ((P, CT, NTOK), "bf16"),
    "HT0": ((P, HH, WW + 2), "bf16"),
    "G0": ((P, NTOK), "bf16"),
    "FT0": ((P, NTOK), "bf16"),
}


def build_program(gelu_mode="hw", dbg=False):
    nc = bacc.Bacc(
        "TRN2",
        target_bir_lowering=False,
        debug=False,
        enable_asserts=False,
        num_devices=N_CORES,
    )
    d = {}
    for name, shape in _SHAPES.items():
        d[name] = nc.dram_tensor(name, list(shape), f32, kind="ExternalInput").ap()
    for name, shape in _BF16_SHAPES.items():
        d[name] = nc.dram_tensor(name, list(shape), bf16, kind="ExternalInput").ap()
    out_ap = nc.dram_tensor("out", [NTOK, C], f32, kind="ExternalOutput").ap()
    dbg_aps = None
    if dbg:
        dbg_aps = {}
        for k, (shape, dt_) in DBG_SPECS.items():
            dbg_aps[k] = nc.dram_tensor(
                f"dbg_{k}", list(shape), bf16 if dt_ == "bf16" else f32,
                kind="ExternalOutput",
            ).ap()
    with tile.TileContext(nc) as tc:
        _emit(tc, d, out_ap, gelu_mode=gelu_mode, dbg=dbg_aps)
    nc.compile()
    return nc


_CACHE = {}
LAST_RESULT = None


def prep_core_inputs(x_b, w):
    """Per-core input map: x_b is this core's [1024, 512] batch slice,
    w the shared host-prepped weight dict."""
    m = {"x": np.ascontiguousarray(x_b, dtype=np.float32)}
    m.update(w)
    return m


def prep_weights(inputs):
    qkv_raw = np.asarray(inputs["qkv_w"], np.float32).T  # [C, 3C], head-interleaved
    # reorder columns to [Qpair0..3 | Kpair0..3 | V(head-major)], folding the
    # 1/sqrt(d) score scale into the q columns
    qkv_wT = np.empty((C, 3 * C), np.float32)
    for h in range(NH):
        qcol = qkv_raw[:, h * 3 * D : h * 3 * D + D] * (D ** -0.5)
        kcol = qkv_raw[:, h * 3 * D + D : h * 3 * D + 2 * D]
        vcol = qkv_raw[:, h * 3 * D + 2 * D : h * 3 * D + 3 * D]
        qkv_wT[:, h * D : (h + 1) * D] = qcol
        qkv_wT[:, C + h * D : C + (h + 1) * D] = kcol
        qkv_wT[:, 2 * C + h * D : 2 * C + (h + 1) * D] = vcol
    w = {
        "qkv_wTb": np.ascontiguousarray(qkv_wT).astype(ml_dtypes.bfloat16),
        "proj_wTb": np.ascontiguousarray(
            np.asarray(inputs["proj_w"], np.float32).T
        ).astype(ml_dtypes.bfloat16),
        "proj_b": np.asarray(inputs["proj_b"], np.float32),
        "ln1_g": np.asarray(inputs["ln1_g"], np.float32),
        "ln1_b": np.asarray(inputs["ln1_b"], np.float32),
        "ln2_g": np.asarray(inputs["ln2_g"], np.float32),
        "ln2_b": np.asarray(inputs["ln2_b"], np.float32),
        "fc1_wTb": np.ascontiguousarray(
            np.asarray(inputs["fc1_w"], np.float32).T
        ).astype(ml_dtypes.bfloat16),
        "fc1_b": np.asarray(inputs["fc1_b"], np.float32),
        "dw_w9": np.ascontiguousarray(
            np.asarray(inputs["dw_w"], np.float32).reshape(HID, 9)
        ),
        "dw_b": np.asarray(inputs["dw_b"], np.float32),
        "fc2_wTb": np.ascontiguousarray(
            np.asarray(inputs["fc2_w"], np.float32).T
        ).astype(ml_dtypes.bfloat16),
        "fc2_b": np.asarray(inputs["fc2_b"], np.float32),
    }
    return w


def kernel(**inputs):
    x = np.asarray(inputs["x"], np.float32)  # [8, 1024, 512]
    assert x.shape == (N_CORES, NTOK, C), x.shape
    w = prep_weights(inputs)
    if "nc" not in _CACHE:
        _CACHE["nc"] = build_program()
    nc = _CACHE["nc"]
    in_maps = [prep_core_inputs(x[i], w) for i in range(N_CORES)]
    res = bass_utils.run_bass_kernel_spmd(nc, in_maps, core_ids=list(range(N_CORES)))
    global LAST_RESULT
    LAST_RESULT = res
    out = np.stack([res.results[i]["out"] for i in range(N_CORES)], axis=0)
    return out.astype(np.float32)
